# revision 1
# baseline (speedup 1.0000x reference)
"""LocalAttention Bass kernel for Trainium2 (8 NeuronCores).

Problem: B=4 H=8 T=8192 D=64, window=128, look_backward=1, causal.
Sharding: pure (B*H) data parallelism — 32 heads / 8 cores = 4 heads each,
processed as 2 head-pairs so q/k DMAs use all 128 SBUF partitions.

Device algorithm (per head, per 128-token window w):
  S^T[k, q] = K_w' @ Q_w^T      (keys on partitions, so the softmax
                                 reduction over keys can ride the PV matmul)
  P = exp(S^T * D^-0.5) * causal01
  [O^T; r] = [V | 1]^T @ P      (ones column baked into V gives row-sums)
Host divides O^T by r and transposes back.

PSUM "pairblock" layout: one [128, 256] matmul per window computes
[T1(w) | T0(w+1)] = K_w vs [Q_w | Q_{w+1}] — 4 matmuls per 4-window group,
all 256-aligned (no PSUM bank crossing). Window w's prev-block T0(w) is
read from the previous group's P tile; the very first window reads a
constant zero tile (its prev window is fully masked padding).

Host-side shard prep (inside kernel(), numpy):
  qTp [2, 128, 8320]  — head-pair Q^T (head A rows 0-63, head B rows
                        64-127), one zero window appended (lookahead pad)
  kT  [2, 128, 8192]  — head-pair K^T
  vp  [4, 128, 4225]  — per head: 65 V slots x [V(64) | 1], slot 0 zeroed
  mask01 [128, 128]   — within-window causal 0/1 (k <= q)
Output:
  outT [4, 65, 8192]  — rows 0..63 unnormalized O^T, row 64 row-sums r
"""

import numpy as np

B, H, T, D = 4, 8, 8192, 64
W = 128                     # window size
WIN = T // W                # 64 windows per head
NCORES = 8
BH = B * H                  # 32
BH_PER_CORE = BH // NCORES  # 4
NPAIR = BH_PER_CORE // 2    # 2 head pairs per core
CHUNK_W = 32                # windows per load chunk
NCHUNK = WIN // CHUNK_W     # 2
G = 4                       # windows per softmax group (PSUM tile = [128, 1024])
SCALE = float(D) ** -0.5

MASK_ON_GPSIMD = True

_nc_cache = {}
last_perf = None


def _build_nc(skip=()):
    import concourse.tile as tile
    from concourse import bacc
    from concourse import mybir
    from contextlib import ExitStack

    f32 = mybir.dt.float32
    Exp = mybir.ActivationFunctionType.Exp
    mult = mybir.AluOpType.mult

    nc = bacc.Bacc()
    qTp = nc.dram_tensor("qTp", [NPAIR, W, (WIN + 1) * W], f32,
                         kind="ExternalInput")
    kT = nc.dram_tensor("kT", [NPAIR, W, T], f32, kind="ExternalInput")
    vp = nc.dram_tensor("vp", [BH_PER_CORE, W, (WIN + 1) * (D + 1)], f32,
                        kind="ExternalInput")
    mask = nc.dram_tensor("mask01", [W, W], f32, kind="ExternalInput")
    outT = nc.dram_tensor("outT", [BH_PER_CORE, D + 1, T], f32,
                          kind="ExternalOutput")

    with tile.TileContext(nc) as tc, ExitStack() as ctx:
        cpool = ctx.enter_context(tc.tile_pool(name="cpool", bufs=1))
        qpool = ctx.enter_context(tc.tile_pool(name="qpool", bufs=2))
        kpool = ctx.enter_context(tc.tile_pool(name="kpool", bufs=2))
        vpool = ctx.enter_context(tc.tile_pool(name="vpool", bufs=4))
        opool = ctx.enter_context(tc.tile_pool(name="opool", bufs=3))
        ppool = ctx.enter_context(tc.tile_pool(name="ppool", bufs=4))
        spsum = ctx.enter_context(tc.tile_pool(name="spsum", bufs=2, space="PSUM"))
        opsum = ctx.enter_context(tc.tile_pool(name="opsum", bufs=3, space="PSUM"))

        mtile = cpool.tile([W, W], f32)
        nc.sync.dma_start(mtile[:], mask[:])
        z128 = cpool.tile([W, W], f32)       # P for the all-masked pad window
        nc.vector.memset(z128[:], 0.0)

        mm = nc.tensor.matmul
        gidx = 0
        for p in range(NPAIR):
            # per-head P tile of the previous group (for cross-group T0 reads)
            pt_prev = [None, None]
            for c in range(NCHUNK):
                c0 = c * CHUNK_W * W
                qc = qpool.tile([W, (CHUNK_W + 1) * W], f32, tag="qc")
                kc = kpool.tile([W, CHUNK_W * W], f32, tag="kc")
                if "loads" not in skip:
                    nc.sync.dma_start(qc[:], qTp[p, :, c0:c0 + (CHUNK_W + 1) * W])
                    nc.sync.dma_start(kc[:], kT[p, :, c0:c0 + CHUNK_W * W])
                vcs = []
                ocs = []
                for h in range(2):
                    vc = vpool.tile([W, (CHUNK_W + 1) * (D + 1)], f32, tag="vc")
                    if "loads" not in skip:
                        v0 = c * CHUNK_W * (D + 1)
                        nc.sync.dma_start(
                            vc[:],
                            vp[2 * p + h, :, v0:v0 + (CHUNK_W + 1) * (D + 1)])
                    vcs.append(vc)
                    oc = opool.tile([D + 1, CHUNK_W * W], f32, tag="oc")
                    ocs.append(oc)

                for g in range(CHUNK_W // G):
                    w0 = g * G
                    for h in range(2):
                        hb = h * 64  # partition base of this head in qc/kc
                        vc = vcs[h]
                        # pairblock matmuls: [T1(w) | T0(w+1)] per window
                        sp = spsum.tile([W, G * 2 * W], f32, tag="sp")
                        if "smm" not in skip:
                            for i in range(G):
                                wl = w0 + i
                                mm(sp[:, i * 256:(i + 1) * 256],
                                   kc[hb:hb + 64, wl * W:(wl + 1) * W],
                                   qc[hb:hb + 64, wl * W:(wl + 2) * W],
                                   start=True, stop=True)

                        pt = ppool.tile([W, G * 2 * W], f32, tag="pt")
                        if "exp" not in skip:
                            nc.scalar.activation(pt[:], sp[:], Exp, scale=SCALE)

                        # causal mask on T1 blocks (cols 0,256,512,768)
                        pt3 = pt[:].rearrange("p (g x) -> p g x", x=2 * W)
                        t1 = pt3[:, :, 0:W]
                        mb = mtile[:, None, :].to_broadcast([W, G, W])
                        if "mask" not in skip:
                            if MASK_ON_GPSIMD and gidx % 2 == 0:
                                nc.gpsimd.tensor_tensor(t1, t1, mb, mult)
                            else:
                                nc.vector.tensor_tensor(t1, t1, mb, mult)

                        # PV + row-sums
                        op = opsum.tile([D + 1, G * W], f32, tag="op")
                        if "pv" not in skip:
                            for i in range(G):
                                wl = w0 + i
                                if i > 0:
                                    t0src = pt[:, i * 256 - W:i * 256]
                                elif pt_prev[h] is not None:
                                    t0src = pt_prev[h][:, G * 256 - W:G * 256]
                                else:
                                    t0src = z128[:]
                                mm(op[:, i * W:(i + 1) * W],
                                   vc[:, wl * (D + 1):(wl + 1) * (D + 1)],
                                   t0src, start=True, stop=False)
                                mm(op[:, i * W:(i + 1) * W],
                                   vc[:, (wl + 1) * (D + 1):(wl + 2) * (D + 1)],
                                   pt[:, i * 256:i * 256 + W],
                                   start=False, stop=True)

                        if "ocopy" not in skip:
                            nc.vector.tensor_copy(
                                ocs[h][:, w0 * W:(w0 + G) * W], op[:])
                        pt_prev[h] = pt
                        gidx += 1

                if "store" not in skip:
                    for h in range(2):
                        nc.sync.dma_start(
                            outT[2 * p + h, :, c0:c0 + CHUNK_W * W], ocs[h][:])
    nc.finalize()
    return nc


def _prep_core_inputs(q2, k2, v2, core):
    s0 = core * BH_PER_CORE
    qTp = np.zeros((NPAIR, W, (WIN + 1) * W), np.float32)
    kTp = np.zeros((NPAIR, W, T), np.float32)
    for p in range(NPAIR):
        for h in range(2):
            bh = s0 + 2 * p + h
            qTp[p, h * 64:(h + 1) * 64, :T] = q2[bh].T
            kTp[p, h * 64:(h + 1) * 64, :] = k2[bh].T
    vr = v2[s0:s0 + BH_PER_CORE].reshape(
        BH_PER_CORE, WIN, W, D).transpose(0, 2, 1, 3)
    vp = np.zeros((BH_PER_CORE, W, WIN + 1, D + 1), np.float32)
    vp[:, :, 1:, :D] = vr
    vp[:, :, :, D] = 1.0
    vp = np.ascontiguousarray(vp.reshape(BH_PER_CORE, W, (WIN + 1) * (D + 1)))
    mask01 = (np.arange(W)[:, None] <= np.arange(W)[None, :]).astype(np.float32)
    return {"qTp": qTp, "kT": kTp, "vp": vp, "mask01": mask01}


def kernel(q, k, v, _trace=False):
    global last_perf
    from concourse.bass_utils import run_bass_kernel_spmd

    q = np.ascontiguousarray(np.asarray(q), dtype=np.float32)
    k = np.ascontiguousarray(np.asarray(k), dtype=np.float32)
    v = np.ascontiguousarray(np.asarray(v), dtype=np.float32)
    q2 = q.reshape(BH, T, D)
    k2 = k.reshape(BH, T, D)
    v2 = v.reshape(BH, T, D)

    if "nc" not in _nc_cache:
        _nc_cache["nc"] = _build_nc()
    nc = _nc_cache["nc"]

    in_maps = [_prep_core_inputs(q2, k2, v2, core) for core in range(NCORES)]
    res = run_bass_kernel_spmd(
        nc, in_maps, core_ids=list(range(NCORES)), trace=_trace)
    last_perf = res

    outs = []
    for core in range(NCORES):
        ot = res.results[core]["outT"]                 # [4, 65, T]
        o = ot[:, :D, :] / ot[:, D:D + 1, :]           # normalize
        outs.append(o.transpose(0, 2, 1))              # [4, T, 64]
    full = np.concatenate(outs, axis=0)                # [32, T, 64]
    return full.reshape(B, H, T, D)



# revision 5
# speedup vs baseline: 1.7821x; 1.7821x over previous
"""LocalAttention Bass kernel for Trainium2 (8 NeuronCores).

Problem: B=4 H=8 T=8192 D=64, window=128, look_backward=1, causal.
Sharding: pure (B*H) data parallelism — 32 heads / 8 cores = 4 heads each,
processed as 2 head-pairs so q/k DMAs use all 128 SBUF partitions.

Device algorithm (per head, per 128-token window w):
  S^T[k, q] = K_w' @ Q_w^T      (keys on partitions, so the softmax
                                 reduction over keys can ride the PV matmul)
  P = exp(S^T * D^-0.5) * causal01
  [O^T; r] = [V | 1]^T @ P      (ones column baked into V gives row-sums)
Host divides O^T by r and transposes back.

PSUM "pairblock" layout: one [128, 256] matmul per window computes
[T1(w) | T0(w+1)] = K_w vs [Q_w | Q_{w+1}] — 4 matmuls per 4-window group,
all 256-aligned (no PSUM bank crossing). Window w's prev-block T0(w) is
read from the previous group's P tile; the very first window reads a
constant zero tile (its prev window is fully masked padding).

Host-side shard prep (inside kernel(), numpy):
  qTp [2, 128, 8320]  — head-pair Q^T (head A rows 0-63, head B rows
                        64-127), one zero window appended (lookahead pad)
  kT  [2, 128, 8192]  — head-pair K^T
  vp  [4, 128, 4225]  — per head: 65 V slots x [V(64) | 1], slot 0 zeroed
  mask01 [128, 128]   — within-window causal 0/1 (k <= q)
Output:
  outT [4, 65, 8192]  — rows 0..63 unnormalized O^T, row 64 row-sums r
"""

import numpy as np

B, H, T, D = 4, 8, 8192, 64
W = 128                     # window size
WIN = T // W                # 64 windows per head
NCORES = 8
BH = B * H                  # 32
BH_PER_CORE = BH // NCORES  # 4
NPAIR = BH_PER_CORE // 2    # 2 head pairs per core
CHUNK_W = 32                # windows per load chunk
NCHUNK = WIN // CHUNK_W     # 2
G = 4                       # windows per softmax group (PSUM tile = [128, 1024])
SCALE = float(D) ** -0.5

MASK_ON_GPSIMD = True

_nc_cache = {}
last_perf = None


def _build_nc(skip=()):
    import concourse.tile as tile
    from concourse import bacc
    from concourse import mybir
    from contextlib import ExitStack

    f32 = mybir.dt.float32
    bf16 = mybir.dt.bfloat16
    Exp = mybir.ActivationFunctionType.Exp
    mult = mybir.AluOpType.mult

    nc = bacc.Bacc()
    qTp = nc.dram_tensor("qTp", [NPAIR, W, (WIN + 1) * W], bf16,
                         kind="ExternalInput")
    kT = nc.dram_tensor("kT", [NPAIR, W, T], bf16, kind="ExternalInput")
    vp = nc.dram_tensor("vp", [BH_PER_CORE, W, (WIN + 1) * (D + 1)], bf16,
                        kind="ExternalInput")
    mask = nc.dram_tensor("mask01", [W, W], bf16, kind="ExternalInput")
    outT = nc.dram_tensor("outT", [BH_PER_CORE, D + 1, T], f32,
                          kind="ExternalOutput")

    with tile.TileContext(nc) as tc, ExitStack() as ctx:
        cpool = ctx.enter_context(tc.tile_pool(name="cpool", bufs=1))
        qpool = ctx.enter_context(tc.tile_pool(name="qpool", bufs=2))
        kpool = ctx.enter_context(tc.tile_pool(name="kpool", bufs=2))
        vpool = ctx.enter_context(tc.tile_pool(name="vpool", bufs=4))
        opool = ctx.enter_context(tc.tile_pool(name="opool", bufs=3))
        ppool = ctx.enter_context(tc.tile_pool(name="ppool", bufs=4))
        spsum = ctx.enter_context(tc.tile_pool(name="spsum", bufs=2, space="PSUM"))
        opsum = ctx.enter_context(tc.tile_pool(name="opsum", bufs=3, space="PSUM"))

        mtile = cpool.tile([W, W], bf16)
        nc.sync.dma_start(mtile[:], mask[:])
        z128 = cpool.tile([W, W], bf16)      # P for the all-masked pad window
        nc.vector.memset(z128[:], 0.0)

        mm = nc.tensor.matmul
        gidx = 0
        for p in range(NPAIR):
            # per-head P tile of the previous group (for cross-group T0 reads)
            pt_prev = [None, None]
            for c in range(NCHUNK):
                c0 = c * CHUNK_W * W
                qc = qpool.tile([W, (CHUNK_W + 1) * W], bf16, tag="qc")
                kc = kpool.tile([W, CHUNK_W * W], bf16, tag="kc")
                if "loads" not in skip:
                    nc.sync.dma_start(qc[:], qTp[p, :, c0:c0 + (CHUNK_W + 1) * W])
                    nc.sync.dma_start(kc[:], kT[p, :, c0:c0 + CHUNK_W * W])
                vcs = []
                ocs = []
                for h in range(2):
                    vc = vpool.tile([W, (CHUNK_W + 1) * (D + 1)], bf16, tag="vc")
                    if "loads" not in skip:
                        v0 = c * CHUNK_W * (D + 1)
                        nc.sync.dma_start(
                            vc[:],
                            vp[2 * p + h, :, v0:v0 + (CHUNK_W + 1) * (D + 1)])
                    vcs.append(vc)
                    oc = opool.tile([D + 1, CHUNK_W * W], f32, tag="oc")
                    ocs.append(oc)

                for g in range(CHUNK_W // G):
                    w0 = g * G
                    for h in range(2):
                        hb = h * 64  # partition base of this head in qc/kc
                        vc = vcs[h]
                        # pairblock matmuls: [T1(w) | T0(w+1)] per window
                        sp = spsum.tile([W, G * 2 * W], f32, tag="sp")
                        if "smm" not in skip:
                            for i in range(G):
                                wl = w0 + i
                                mm(sp[:, i * 256:(i + 1) * 256],
                                   kc[hb:hb + 64, wl * W:(wl + 1) * W],
                                   qc[hb:hb + 64, wl * W:(wl + 2) * W],
                                   start=True, stop=True)

                        pt = ppool.tile([W, G * 2 * W], bf16, tag="pt")
                        if "exp" not in skip:
                            nc.scalar.activation(pt[:], sp[:], Exp, scale=SCALE)

                        # causal mask on T1 blocks (cols 0,256,512,768)
                        pt3 = pt[:].rearrange("p (g x) -> p g x", x=2 * W)
                        t1 = pt3[:, :, 0:W]
                        mb = mtile[:, None, :].to_broadcast([W, G, W])
                        if "mask" not in skip:
                            if MASK_ON_GPSIMD and gidx % 2 == 0:
                                nc.gpsimd.tensor_tensor(t1, t1, mb, mult)
                            else:
                                nc.vector.tensor_tensor(t1, t1, mb, mult)

                        # PV + row-sums
                        op = opsum.tile([D + 1, G * W], f32, tag="op")
                        if "pv" not in skip:
                            for i in range(G):
                                wl = w0 + i
                                if i > 0:
                                    t0src = pt[:, i * 256 - W:i * 256]
                                elif pt_prev[h] is not None:
                                    t0src = pt_prev[h][:, G * 256 - W:G * 256]
                                else:
                                    t0src = z128[:]
                                mm(op[:, i * W:(i + 1) * W],
                                   vc[:, wl * (D + 1):(wl + 1) * (D + 1)],
                                   t0src, start=True, stop=False)
                                mm(op[:, i * W:(i + 1) * W],
                                   vc[:, (wl + 1) * (D + 1):(wl + 2) * (D + 1)],
                                   pt[:, i * 256:i * 256 + W],
                                   start=False, stop=True)

                        if "ocopy" not in skip:
                            nc.vector.tensor_copy(
                                ocs[h][:, w0 * W:(w0 + G) * W], op[:])
                        pt_prev[h] = pt
                        gidx += 1

                if "store" not in skip:
                    for h in range(2):
                        nc.sync.dma_start(
                            outT[2 * p + h, :, c0:c0 + CHUNK_W * W], ocs[h][:])
    nc.finalize()
    return nc


def _prep_core_inputs(q2, k2, v2, core):
    from ml_dtypes import bfloat16
    s0 = core * BH_PER_CORE
    qTp = np.zeros((NPAIR, W, (WIN + 1) * W), bfloat16)
    kTp = np.zeros((NPAIR, W, T), bfloat16)
    for p in range(NPAIR):
        for h in range(2):
            bh = s0 + 2 * p + h
            qTp[p, h * 64:(h + 1) * 64, :T] = q2[bh].T
            kTp[p, h * 64:(h + 1) * 64, :] = k2[bh].T
    vr = v2[s0:s0 + BH_PER_CORE].reshape(
        BH_PER_CORE, WIN, W, D).transpose(0, 2, 1, 3)
    vp = np.zeros((BH_PER_CORE, W, WIN + 1, D + 1), bfloat16)
    vp[:, :, 1:, :D] = vr
    vp[:, :, :, D] = 1.0
    vp = np.ascontiguousarray(vp.reshape(BH_PER_CORE, W, (WIN + 1) * (D + 1)))
    mask01 = (np.arange(W)[:, None] <= np.arange(W)[None, :]).astype(bfloat16)
    return {"qTp": qTp, "kT": kTp, "vp": vp, "mask01": mask01}


def kernel(q, k, v, _trace=False):
    global last_perf
    from concourse.bass_utils import run_bass_kernel_spmd

    q = np.ascontiguousarray(np.asarray(q), dtype=np.float32)
    k = np.ascontiguousarray(np.asarray(k), dtype=np.float32)
    v = np.ascontiguousarray(np.asarray(v), dtype=np.float32)
    q2 = q.reshape(BH, T, D)
    k2 = k.reshape(BH, T, D)
    v2 = v.reshape(BH, T, D)

    if "nc" not in _nc_cache:
        _nc_cache["nc"] = _build_nc()
    nc = _nc_cache["nc"]

    in_maps = [_prep_core_inputs(q2, k2, v2, core) for core in range(NCORES)]
    res = run_bass_kernel_spmd(
        nc, in_maps, core_ids=list(range(NCORES)), trace=_trace)
    last_perf = res

    outs = []
    for core in range(NCORES):
        ot = res.results[core]["outT"]                 # [4, 65, T]
        o = ot[:, :D, :] / ot[:, D:D + 1, :]           # normalize
        outs.append(o.transpose(0, 2, 1))              # [4, T, 64]
    full = np.concatenate(outs, axis=0)                # [32, T, 64]
    return full.reshape(B, H, T, D)



# revision 7
# speedup vs baseline: 2.2105x; 1.2404x over previous
"""LocalAttention Bass kernel for Trainium2 (8 NeuronCores).

Problem: B=4 H=8 T=8192 D=64, window=128, look_backward=1, causal.
Sharding: pure (B*H) data parallelism — 32 heads / 8 cores = 4 heads each,
processed as 2 head-pairs so q/k DMAs use all 128 SBUF partitions.

Device algorithm (per head, per 128-token window w):
  S^T[k, q] = K_w' @ Q_w^T      (keys on partitions, so the softmax
                                 reduction over keys can ride the PV matmul)
  P = exp(S^T * D^-0.5) * causal01
  [O^T; r] = [V | 1]^T @ P      (ones column baked into V gives row-sums)
Host divides O^T by r and transposes back.

PSUM "pairblock" layout: one [128, 256] matmul per window computes
[T1(w) | T0(w+1)] = K_w vs [Q_w | Q_{w+1}] — 4 matmuls per 4-window group,
all 256-aligned (no PSUM bank crossing). Window w's prev-block T0(w) is
read from the previous group's P tile; the very first window reads a
constant zero tile (its prev window is fully masked padding).

Host-side shard prep (inside kernel(), numpy):
  qTp [2, 128, 8320]  — head-pair Q^T (head A rows 0-63, head B rows
                        64-127), one zero window appended (lookahead pad)
  kT  [2, 128, 8192]  — head-pair K^T
  vp  [4, 128, 4225]  — per head: 65 V slots x [V(64) | 1], slot 0 zeroed
  mask01 [128, 128]   — within-window causal 0/1 (k <= q)
Output:
  outT [4, 65, 8192]  — rows 0..63 unnormalized O^T, row 64 row-sums r
"""

import numpy as np

B, H, T, D = 4, 8, 8192, 64
W = 128                     # window size
WIN = T // W                # 64 windows per head
NCORES = 8
BH = B * H                  # 32
BH_PER_CORE = BH // NCORES  # 4
NPAIR = BH_PER_CORE // 2    # 2 head pairs per core
CHUNK_W = 32                # windows per load chunk
NCHUNK = WIN // CHUNK_W     # 2
G = 4                       # windows per softmax group (PSUM tile = [128, 1024])
SCALE = float(D) ** -0.5

MASK_ON_GPSIMD = True

_nc_cache = {}
last_perf = None


def _build_nc(skip=()):
    import concourse.tile as tile
    from concourse import bacc
    from concourse import mybir
    from contextlib import ExitStack

    f32 = mybir.dt.float32
    bf16 = mybir.dt.bfloat16
    Exp = mybir.ActivationFunctionType.Exp
    mult = mybir.AluOpType.mult

    nc = bacc.Bacc()
    qTp = nc.dram_tensor("qTp", [NPAIR, W, (WIN + 1) * W], bf16,
                         kind="ExternalInput")
    kT = nc.dram_tensor("kT", [NPAIR, W, T], bf16, kind="ExternalInput")
    vp = nc.dram_tensor("vp", [BH_PER_CORE, W, (WIN + 1) * (D + 1)], bf16,
                        kind="ExternalInput")
    mask = nc.dram_tensor("mask01", [W, W], bf16, kind="ExternalInput")
    outT = nc.dram_tensor("outT", [BH_PER_CORE, D + 1, T], f32,
                          kind="ExternalOutput")

    with tile.TileContext(nc) as tc, ExitStack() as ctx:
        cpool = ctx.enter_context(tc.tile_pool(name="cpool", bufs=1))
        qpool = ctx.enter_context(tc.tile_pool(name="qpool", bufs=2))
        kpool = ctx.enter_context(tc.tile_pool(name="kpool", bufs=2))
        vpool = ctx.enter_context(tc.tile_pool(name="vpool", bufs=4))
        opool = ctx.enter_context(tc.tile_pool(name="opool", bufs=3))
        ppool = ctx.enter_context(tc.tile_pool(name="ppool", bufs=4))
        spsum = ctx.enter_context(tc.tile_pool(name="spsum", bufs=2, space="PSUM"))
        opsum = ctx.enter_context(tc.tile_pool(name="opsum", bufs=3, space="PSUM"))

        mtile = cpool.tile([W, W], bf16)
        nc.sync.dma_start(mtile[:], mask[:])
        z128 = cpool.tile([W, W], bf16)      # P for the all-masked pad window
        nc.vector.memset(z128[:], 0.0)

        mm = nc.tensor.matmul
        gidx = 0
        for p in range(NPAIR):
            # per-head P tile of the previous group (for cross-group T0 reads)
            pt_prev = [None, None]
            for c in range(NCHUNK):
                c0 = c * CHUNK_W * W
                qc = qpool.tile([W, (CHUNK_W + 1) * W], bf16, tag="qc")
                kc = kpool.tile([W, CHUNK_W * W], bf16, tag="kc")
                first = p == 0 and c == 0
                if "loads" not in skip:
                    if first:
                        # split so the first groups' data lands sooner
                        s = 9 * W
                        nc.sync.dma_start(qc[:, :s], qTp[p, :, c0:c0 + s])
                        nc.sync.dma_start(kc[:, :s], kT[p, :, c0:c0 + s])
                        nc.sync.dma_start(
                            qc[:, s:], qTp[p, :, c0 + s:c0 + (CHUNK_W + 1) * W])
                        nc.sync.dma_start(
                            kc[:, s:], kT[p, :, c0 + s:c0 + CHUNK_W * W])
                    else:
                        nc.sync.dma_start(
                            qc[:], qTp[p, :, c0:c0 + (CHUNK_W + 1) * W])
                        nc.sync.dma_start(kc[:], kT[p, :, c0:c0 + CHUNK_W * W])
                vcs = []
                ocs = []
                for h in range(2):
                    vc = vpool.tile([W, (CHUNK_W + 1) * (D + 1)], bf16, tag="vc")
                    if "loads" not in skip:
                        v0 = c * CHUNK_W * (D + 1)
                        sv = 9 * (D + 1)
                        if first:
                            nc.gpsimd.dma_start(
                                vc[:, :sv], vp[2 * p + h, :, v0:v0 + sv])
                            nc.gpsimd.dma_start(
                                vc[:, sv:],
                                vp[2 * p + h,
                                   :, v0 + sv:v0 + (CHUNK_W + 1) * (D + 1)])
                        else:
                            nc.gpsimd.dma_start(
                                vc[:],
                                vp[2 * p + h, :, v0:v0 + (CHUNK_W + 1) * (D + 1)])
                    vcs.append(vc)
                    oc = opool.tile([D + 1, CHUNK_W * W], f32, tag="oc")
                    ocs.append(oc)

                for g in range(CHUNK_W // G):
                    w0 = g * G
                    for h in range(2):
                        hb = h * 64  # partition base of this head in qc/kc
                        vc = vcs[h]
                        # pairblock matmuls: [T1(w) | T0(w+1)] per window
                        sp = spsum.tile([W, G * 2 * W], f32, tag="sp")
                        if "smm" not in skip:
                            for i in range(G):
                                wl = w0 + i
                                mm(sp[:, i * 256:(i + 1) * 256],
                                   kc[hb:hb + 64, wl * W:(wl + 1) * W],
                                   qc[hb:hb + 64, wl * W:(wl + 2) * W],
                                   start=True, stop=True)

                        pt = ppool.tile([W, G * 2 * W], bf16, tag="pt")
                        if "exp" not in skip:
                            nc.scalar.activation(pt[:], sp[:], Exp, scale=SCALE)

                        # causal mask on T1 blocks (cols 0,256,512,768)
                        pt3 = pt[:].rearrange("p (g x) -> p g x", x=2 * W)
                        t1 = pt3[:, :, 0:W]
                        mb = mtile[:, None, :].to_broadcast([W, G, W])
                        if "mask" not in skip:
                            nc.vector.tensor_tensor(t1, t1, mb, mult)

                        # PV + row-sums: per window pair (wl, wl+1):
                        #   MM1 256-wide: slot wl+1 x pairblock(wl)
                        #     -> T1(wl) into cols wl, T0(wl+1) into cols wl+1
                        #   MM2 128-wide: slot wl   x T0(wl)   -> cols wl
                        #   MM3 128-wide: slot wl+2 x T1(wl+1) -> cols wl+1
                        op = opsum.tile([D + 1, G * W], f32, tag="op")
                        if "pv" not in skip:
                            for i in range(0, G, 2):
                                wl = w0 + i
                                vs = lambda s: vc[:, s * (D + 1):(s + 1) * (D + 1)]
                                if i > 0:
                                    t0src = pt[:, i * 256 - W:i * 256]
                                elif pt_prev[h] is not None:
                                    t0src = pt_prev[h][:, G * 256 - W:G * 256]
                                else:
                                    t0src = z128[:]
                                mm(op[:, i * W:(i + 2) * W],
                                   vs(wl + 1), pt[:, i * 256:(i + 1) * 256],
                                   start=True, stop=False)
                                mm(op[:, i * W:(i + 1) * W],
                                   vs(wl), t0src,
                                   start=False, stop=True, skip_group_check=True)
                                mm(op[:, (i + 1) * W:(i + 2) * W],
                                   vs(wl + 2),
                                   pt[:, (i + 1) * 256:(i + 1) * 256 + W],
                                   start=False, stop=True, skip_group_check=True)

                        if "ocopy" not in skip:
                            nc.vector.tensor_copy(
                                ocs[h][:, w0 * W:(w0 + G) * W], op[:])
                        pt_prev[h] = pt
                        gidx += 1

                    # store first half of the chunk early to overlap the tail
                    if "store" not in skip and g == CHUNK_W // G // 2 - 1:
                        half = CHUNK_W * W // 2
                        for h in range(2):
                            nc.gpsimd.dma_start(
                                outT[2 * p + h, :, c0:c0 + half],
                                ocs[h][:, :half])

                if "store" not in skip:
                    half = CHUNK_W * W // 2
                    for h in range(2):
                        nc.gpsimd.dma_start(
                            outT[2 * p + h, :, c0 + half:c0 + CHUNK_W * W],
                            ocs[h][:, half:])
    nc.finalize()
    return nc


def _prep_core_inputs(q2, k2, v2, core):
    from ml_dtypes import bfloat16
    s0 = core * BH_PER_CORE
    qTp = np.zeros((NPAIR, W, (WIN + 1) * W), bfloat16)
    kTp = np.zeros((NPAIR, W, T), bfloat16)
    for p in range(NPAIR):
        for h in range(2):
            bh = s0 + 2 * p + h
            qTp[p, h * 64:(h + 1) * 64, :T] = q2[bh].T
            kTp[p, h * 64:(h + 1) * 64, :] = k2[bh].T
    vr = v2[s0:s0 + BH_PER_CORE].reshape(
        BH_PER_CORE, WIN, W, D).transpose(0, 2, 1, 3)
    vp = np.zeros((BH_PER_CORE, W, WIN + 1, D + 1), bfloat16)
    vp[:, :, 1:, :D] = vr
    vp[:, :, :, D] = 1.0
    vp = np.ascontiguousarray(vp.reshape(BH_PER_CORE, W, (WIN + 1) * (D + 1)))
    mask01 = (np.arange(W)[:, None] <= np.arange(W)[None, :]).astype(bfloat16)
    return {"qTp": qTp, "kT": kTp, "vp": vp, "mask01": mask01}


def kernel(q, k, v, _trace=False):
    global last_perf
    from concourse.bass_utils import run_bass_kernel_spmd

    q = np.ascontiguousarray(np.asarray(q), dtype=np.float32)
    k = np.ascontiguousarray(np.asarray(k), dtype=np.float32)
    v = np.ascontiguousarray(np.asarray(v), dtype=np.float32)
    q2 = q.reshape(BH, T, D)
    k2 = k.reshape(BH, T, D)
    v2 = v.reshape(BH, T, D)

    if "nc" not in _nc_cache:
        _nc_cache["nc"] = _build_nc()
    nc = _nc_cache["nc"]

    in_maps = [_prep_core_inputs(q2, k2, v2, core) for core in range(NCORES)]
    res = run_bass_kernel_spmd(
        nc, in_maps, core_ids=list(range(NCORES)), trace=_trace)
    last_perf = res

    outs = []
    for core in range(NCORES):
        ot = res.results[core]["outT"]                 # [4, 65, T]
        o = ot[:, :D, :] / ot[:, D:D + 1, :]           # normalize
        outs.append(o.transpose(0, 2, 1))              # [4, T, 64]
    full = np.concatenate(outs, axis=0)                # [32, T, 64]
    return full.reshape(B, H, T, D)

